# revision 34
# baseline (speedup 1.0000x reference)
"""Causal multi-head attention layer (train forward) on 8 Trainium2 NeuronCores.

Sharding: batch (4) x head-group (2 of 8 heads each) -> 8 cores.
Per core (batch b, head group g): project Q^T/K^T [512,S] and V [S,512] from
x_b in bf16 (fp32 PSUM accum), run causal attention head-pair-packed on the PE
array (row tiles at partitions 0/64, one [128,1024] PSUM strip pair per key
chunk), softmax rowsums ride a ones column on V (ctx matmul M=65), the
normalization reciprocal is batched per window on DVE and applied during the
ctx drain, then a partial output projection with this core's Wo row block.
Host pre-casts weights/x to bf16, sums the two partials per batch, adds bo.
"""
import numpy as np
import ml_dtypes

import concourse.bass as bass
import concourse.tile as tile
from concourse import bacc, mybir
from concourse.bass_utils import run_bass_kernel_spmd

F32 = mybir.dt.float32
BF16 = mybir.dt.bfloat16
AF = mybir.ActivationFunctionType
ALU = mybir.AluOpType

P = 128
D = 1024          # model dim
DC = 512          # per-core head dims (8 heads x 64)
HD = 64
NHC = 8           # heads per core
NPAIR = 4         # head pairs per core
FC = D // P       # 8 feature chunks
OC = DC // P      # 4 outdim chunks (= head pairs)
W = 512           # query window (fp32 PSUM bank)
WT = W // P       # token chunks per window
SCALE = 1.0 / 32.0  # 1/sqrt(D)


def _copy(nc, i, out, in_, dve_only=False):
    if dve_only or i % 2 == 0:
        nc.vector.tensor_copy(out, in_)
    else:
        nc.scalar.copy(out, in_)


def build_nc(S=2048, num_devices=8, with_bv=False):
    NWIN = S // W

    nc = bacc.Bacc("TRN2", target_bir_lowering=False, debug=False,
                   num_devices=num_devices)
    x = nc.dram_tensor("x", [S, D], BF16, kind="ExternalInput").ap()
    wq = nc.dram_tensor("wq", [D, DC], BF16, kind="ExternalInput").ap()
    wk = nc.dram_tensor("wk", [D, DC], BF16, kind="ExternalInput").ap()
    wv = nc.dram_tensor("wv", [D, DC], BF16, kind="ExternalInput").ap()
    wo = nc.dram_tensor("wo", [DC, D], BF16, kind="ExternalInput").ap()
    bq = nc.dram_tensor("bq", [DC], F32, kind="ExternalInput").ap()
    bk = nc.dram_tensor("bk", [DC], F32, kind="ExternalInput").ap()
    bv = nc.dram_tensor("bv", [DC], F32, kind="ExternalInput").ap()
    tri = nc.dram_tensor("tri", [P, P], BF16, kind="ExternalInput").ap()
    out = nc.dram_tensor("out", [S, D], F32, kind="ExternalOutput").ap()

    with tile.TileContext(nc) as tc:
        with tc.tile_pool(name="const", bufs=1) as cst, \
             tc.tile_pool(name="stage", bufs=3) as stg, \
             tc.tile_pool(name="pt", bufs=5) as ptp, \
             tc.tile_pool(name="small", bufs=2) as sml, \
             tc.tile_pool(name="stgp", bufs=2) as stgp, \
             tc.tile_pool(name="psA", bufs=1, space="PSUM") as psA, \
             tc.tile_pool(name="psC", bufs=1, space="PSUM") as psC:

            mm_ctr = [0]

            def mm_tile(dt=F32):
                i = mm_ctr[0]
                mm_ctr[0] += 1
                return psA.tile([P, 1024], dt, tag=f"s{i % 3}",
                                name=f"mm_s{i % 3}")

            # --- constants (already bf16 from host) ---
            tri_bf = cst.tile([P, P], BF16, tag="tri")
            bq_sb = cst.tile([P, OC], F32, tag="bq")
            bk_sb = cst.tile([P, OC], F32, tag="bk")
            bv_sb = cst.tile([P, OC], F32, tag="bv")
            w_sbs = {}
            for name in ("wq", "wk", "wv"):
                w_sbs[name] = cst.tile([P, FC, DC], BF16, tag=name, name=name)
            wo_sb = cst.tile([P, OC, D], BF16, tag="wo")

            def emit_weights():
                nc.gpsimd.dma_start(tri_bf[:], tri[:])
                nc.gpsimd.dma_start(bq_sb[:],
                                    bq.rearrange("(c p) -> p c", p=P))
                nc.gpsimd.dma_start(bk_sb[:],
                                    bk.rearrange("(c p) -> p c", p=P))
                nc.gpsimd.dma_start(bv_sb[:],
                                    bv.rearrange("(c p) -> p c", p=P))
                for wi, (name, wdram) in enumerate(
                        (("wq", wq), ("wk", wk), ("wv", wv))):
                    w_sb = w_sbs[name]
                    for fc in range(FC):
                        wq_ = nc.sync if (wi + fc) % 2 == 0 else nc.gpsimd
                        wq_.dma_start(w_sb[:, fc, :],
                                      wdram[fc * P:(fc + 1) * P, :])
                for c in range(OC):
                    wq_ = nc.sync if c % 2 == 0 else nc.gpsimd
                    wq_.dma_start(wo_sb[:, c, :], wo[c * P:(c + 1) * P, :])

            # --- per-window tiles ---
            xT_w, qT_w, kT_w, v_w, ctx_w = [], [], [], [], []
            for j in range(NWIN):
                xT_w.append(cst.tile([P, FC, W], BF16, tag=f"xT{j}",
                                     name=f"xT{j}"))
                qT_w.append(cst.tile([P, OC, W], BF16, tag=f"qT{j}",
                                     name=f"qT{j}"))
                kT_w.append(cst.tile([P, OC, W], BF16, tag=f"kT{j}",
                                     name=f"kT{j}"))
                v_w.append(cst.tile([P, WT, NHC, HD + 1], BF16, tag=f"v{j}",
                                    name=f"v{j}"))
                ctx_w.append(cst.tile([P, NPAIR, W], BF16, tag=f"ctx{j}",
                                      name=f"ctx{j}"))
                nc.vector.memset(v_w[j][:, :, :, HD:HD + 1], 1.0)

            eng_ctr = [0]

            def nxt():
                eng_ctr[0] += 1
                return eng_ctr[0]

            def emit_xt(j):
                # --- x^T window j: XBAR DMA transpose straight from DRAM ---
                for t in range(WT):
                    tokc = j * WT + t
                    nc.sync.dma_start_transpose(
                        xT_w[j][:, :, t * P:(t + 1) * P],
                        x[tokc * P:(tokc + 1) * P, :])

            def emit_proj(j, with_xt=True):
                late = j >= 2
                if with_xt:
                    emit_xt(j)

                # --- Q^T / K^T window j (2 outdim chunks per psum slot) ---
                for dst, wname, b_sb in ((qT_w[j], "wq", bq_sb),
                                         (kT_w[j], "wk", bk_sb)):
                    w_sb = w_sbs[wname]
                    for og in range(OC // 2):
                        ps = mm_tile()
                        for half in range(2):
                            oc = og * 2 + half
                            for fc in range(FC):
                                nc.tensor.matmul(
                                    ps[:, half * W:(half + 1) * W],
                                    w_sb[:, fc, oc * P:(oc + 1) * P],
                                    xT_w[j][:, fc, :],
                                    start=(fc == 0), stop=(fc == FC - 1))
                        for half in range(2):
                            oc = og * 2 + half
                            hv = ps[:, half * W:(half + 1) * W]
                            if nxt() % 2 == 0:
                                nc.vector.tensor_scalar(
                                    dst[:, oc, :], hv,
                                    b_sb[:, oc:oc + 1], None, ALU.add)
                            else:
                                nc.scalar.activation(
                                    dst[:, oc, :], hv, AF.Identity,
                                    bias=b_sb[:, oc:oc + 1])

                # --- V window j (2 token chunks per psum slot) ---
                for tg in range(WT // 2):
                    ps = mm_tile()
                    for half in range(2):
                        t = tg * 2 + half
                        for fc in range(FC):
                            nc.tensor.matmul(
                                ps[:, half * W:(half + 1) * W],
                                xT_w[j][:, fc, t * P:(t + 1) * P],
                                w_sbs["wv"][:, fc, :],
                                start=(fc == 0), stop=(fc == FC - 1))
                    dv = ps.rearrange("p (t h n) -> p t h n", t=2, h=NHC)
                    _copy(nc, nxt(), v_w[j][:, tg * 2:tg * 2 + 2, :, 0:HD], dv)

            def emit_attention(j, pairs, stgw=None, rsw=None):
                # --- attention for the given head pairs, window j ---
                skc_hi = WT * (j + 1)
                if stgw is None:
                    stgw = stgp.tile([P, NPAIR, W], BF16, tag="stgw",
                                     name="stgw")
                    rsw = sml.tile([NHC, W], F32, tag="rsw", name="rsw")
                for p in pairs:
                    ctx0 = psC.tile([P, W], F32, tag="c0", name="ctx0")
                    ctx1 = psC.tile([P, W], F32, tag="c1", name="ctx1")
                    for skc in range(skc_hi):
                        jk, tk = divmod(skc, WT)
                        rel = skc * P - j * W
                        vs = max(rel, 0)
                        sp = mm_tile()
                        spv = sp.rearrange("p (h n) -> p h n", h=2)
                        nc.tensor.matmul(sp[:, vs:W],
                                         kT_w[jk][0:HD, p, tk * P:(tk + 1) * P],
                                         qT_w[j][0:HD, p, vs:W],
                                         start=True, stop=True)
                        nc.tensor.matmul(sp[:, W + vs:2 * W],
                                         kT_w[jk][HD:P, p, tk * P:(tk + 1) * P],
                                         qT_w[j][HD:P, p, vs:W],
                                         start=True, stop=True)
                        pt = ptp.tile([P, 1024], BF16, tag="pt", name="pt")
                        ptv = pt.rearrange("p (h n) -> p h n", h=2)
                        nc.scalar.activation(ptv[:, :, vs:W], spv[:, :, vs:W],
                                             AF.Exp, scale=SCALE)
                        if rel >= 0:
                            nc.vector.tensor_tensor(
                                ptv[:, :, rel:rel + P], ptv[:, :, rel:rel + P],
                                tri_bf[:, None, :].to_broadcast([P, 2, P]),
                                ALU.mult)
                        st0 = (skc == 0)
                        sp0 = (skc == skc_hi - 1)
                        nc.tensor.matmul(ctx0[0:HD + 1, vs:W],
                                         v_w[jk][:, tk, 2 * p, :],
                                         ptv[:, 0, vs:W], start=st0, stop=sp0)
                        nc.tensor.matmul(ctx1[0:HD + 1, vs:W],
                                         v_w[jk][:, tk, 2 * p + 1, :],
                                         ptv[:, 1, vs:W], start=st0, stop=sp0)

                    # fast drains (pair layout); rowsums to collector
                    prel = p - pairs[0]
                    for h, ctxp in ((0, ctx0), (1, ctx1)):
                        rw = sml.tile([1, W], F32, tag=f"rw{h}", name="rw")
                        nc.vector.tensor_copy(rw[:], ctxp[HD:HD + 1, :])
                        ri = 2 * prel + h
                        rq = nc.sync if (p + h) % 2 == 0 else nc.gpsimd
                        rq.dma_start(rsw[ri:ri + 1, :], rw[:])
                        _copy(nc, nxt(), stgw[h * HD:(h + 1) * HD, prel, :],
                              ctxp[0:HD, :])

                return stgw, rsw

            def emit_norm(j, stgw, rsw, pairs):
                # batched reciprocal + normalization for the given pairs
                nr = 2 * len(pairs)
                rcf = sml.tile([NHC, W], F32, tag="rcf", name="rcf")
                nc.vector.reciprocal(rcf[0:nr, :], rsw[0:nr, :])
                rcw = sml.tile([NHC, W], BF16, tag="rcw", name="rcw")
                nc.vector.tensor_copy(rcw[0:nr, :], rcf[0:nr, :])
                for p in pairs:
                    prel = p - pairs[0]
                    bc = sml.tile([P, W], BF16, tag=f"bc{p % 2}", name="bc")
                    bq_ = nc.sync if p % 2 == 0 else nc.gpsimd
                    bq_.dma_start(
                        bc[:], rcw[2 * prel:2 * prel + 2, None,
                                   :].to_broadcast([2, HD, W]))
                    for h in range(2):
                        sl = slice(h * HD, (h + 1) * HD)
                        dst = ctx_w[j][sl, p, :]
                        nc.vector.tensor_tensor(dst, stgw[sl, prel, :],
                                                bc[sl, :], ALU.mult)
                        if with_bv:
                            nc.vector.tensor_scalar(
                                dst, dst, bv_sb[sl, p:p + 1], None, ALU.add)

            def emit_outproj(j):
                # --- output projection for window j's tokens ---
                for t in range(WT):
                    tokc = j * WT + t
                    ps = mm_tile()
                    for nb in range(2):
                        for pr in range(NPAIR):
                            nc.tensor.matmul(
                                ps[:, nb * W:(nb + 1) * W],
                                ctx_w[j][:, pr, t * P:(t + 1) * P],
                                wo_sb[:, pr, nb * 512:(nb + 1) * 512],
                                start=(pr == 0), stop=(pr == NPAIR - 1))
                    ost = stg.tile([P, D], F32, tag="ostage")
                    _copy(nc, nxt(), ost[:], ps[:])
                    oeng = nc.sync if t % 2 == 0 else nc.gpsimd
                    oeng.dma_start(out[tokc * P:(tokc + 1) * P, :], ost[:])

            emit_xt(0)
            emit_weights()
            emit_proj(0, with_xt=False)
            for j in range(NWIN):
                if j + 1 < NWIN:
                    stgw, rsw = emit_attention(j, list(range(NPAIR)))
                    emit_proj(j + 1)
                    emit_norm(j, stgw, rsw, list(range(NPAIR)))
                else:
                    units = [emit_attention(j, [p]) + ([p],)
                             for p in range(2)]
                    for p in range(2, NPAIR):
                        u = emit_attention(j, [p])
                        emit_norm(j, *units.pop(0))
                        units.append(u + ([p],))
                    for u in units:
                        emit_norm(j, *u)
                emit_outproj(j)

    nc.compile()
    return nc


def make_in_maps(x, Wq, bq, Wk, bk, Wv, bv, Wo):
    BF = ml_dtypes.bfloat16
    # tri[p, f] = 1 where f >= p (keep key p for query f within a diag block)
    tri = np.triu(np.ones((P, P), dtype=np.float32)).astype(BF)
    in_maps = []
    for c in range(8):
        b, g = c // 2, c % 2
        sl = slice(g * DC, (g + 1) * DC)
        in_maps.append({
            "x": np.ascontiguousarray(x[b]).astype(BF),
            "wq": np.ascontiguousarray(Wq[:, sl]).astype(BF),
            "wk": np.ascontiguousarray(Wk[:, sl]).astype(BF),
            "wv": np.ascontiguousarray(Wv[:, sl]).astype(BF),
            "wo": np.ascontiguousarray(Wo[sl, :]).astype(BF),
            "bq": np.ascontiguousarray(bq[sl]),
            "bk": np.ascontiguousarray(bk[sl]),
            "bv": np.ascontiguousarray(bv[sl]),
            "tri": tri,
        })
    return in_maps


_NC_CACHE = {}


def kernel(x, Wq, bq, Wk, bk, Wv, bv, Wo, bo):
    x = np.asarray(x, dtype=np.float32)
    args = [np.asarray(a, dtype=np.float32)
            for a in (Wq, bq, Wk, bk, Wv, bv, Wo, bo)]
    Wq, bq, Wk, bk, Wv, bv, Wo, bo = args
    key = ("nc", x.shape[1], bool(np.any(bv)))
    if key not in _NC_CACHE:
        _NC_CACHE[key] = build_nc(S=x.shape[1], num_devices=8,
                                  with_bv=bool(np.any(bv)))
    nc = _NC_CACHE[key]
    in_maps = make_in_maps(x, Wq, bq, Wk, bk, Wv, bv, Wo)
    res = run_bass_kernel_spmd(nc, in_maps, core_ids=list(range(8)))
    B = x.shape[0]
    out = np.empty_like(x)
    for b in range(B):
        out[b] = res.results[2 * b]["out"] + res.results[2 * b + 1]["out"] + bo
    return out


# revision 35
# speedup vs baseline: 1.0790x; 1.0790x over previous
"""Causal multi-head attention layer (train forward) on 8 Trainium2 NeuronCores.

Sharding: batch (4) x head-group (2 of 8 heads each) -> 8 cores.
Per core (batch b, head group g): project Q^T/K^T [512,S] and V [S,512] from
x_b in bf16 (fp32 PSUM accum), run causal attention head-pair-packed on the PE
array (row tiles at partitions 0/64, one [128,1024] PSUM strip pair per key
chunk), softmax rowsums ride a ones column on V (ctx matmul M=65), the
normalization reciprocal is batched per window on DVE and applied during the
ctx drain, then a partial output projection with this core's Wo row block.
Host pre-casts weights/x to bf16, sums the two partials per batch, adds bo.
"""
import numpy as np
import ml_dtypes

import concourse.bass as bass
import concourse.tile as tile
from concourse import bacc, mybir
from concourse.bass_utils import run_bass_kernel_spmd

F32 = mybir.dt.float32
BF16 = mybir.dt.bfloat16
AF = mybir.ActivationFunctionType
ALU = mybir.AluOpType

P = 128
D = 1024          # model dim
DC = 512          # per-core head dims (8 heads x 64)
HD = 64
NHC = 8           # heads per core
NPAIR = 4         # head pairs per core
FC = D // P       # 8 feature chunks
OC = DC // P      # 4 outdim chunks (= head pairs)
W = 512           # query window (fp32 PSUM bank)
WT = W // P       # token chunks per window
SCALE = 1.0 / 32.0  # 1/sqrt(D)


def _copy(nc, i, out, in_, dve_only=False):
    if dve_only or i % 2 == 0:
        nc.vector.tensor_copy(out, in_)
    else:
        nc.scalar.copy(out, in_)


def build_nc(S=2048, num_devices=8, with_bv=False):
    NWIN = S // W

    nc = bacc.Bacc("TRN2", target_bir_lowering=False, debug=False,
                   num_devices=num_devices)
    x = nc.dram_tensor("x", [S, D], BF16, kind="ExternalInput").ap()
    wq = nc.dram_tensor("wq", [D, DC], BF16, kind="ExternalInput").ap()
    wk = nc.dram_tensor("wk", [D, DC], BF16, kind="ExternalInput").ap()
    wv = nc.dram_tensor("wv", [D, DC], BF16, kind="ExternalInput").ap()
    wo = nc.dram_tensor("wo", [DC, D], BF16, kind="ExternalInput").ap()
    bq = nc.dram_tensor("bq", [DC], F32, kind="ExternalInput").ap()
    bk = nc.dram_tensor("bk", [DC], F32, kind="ExternalInput").ap()
    bv = nc.dram_tensor("bv", [DC], F32, kind="ExternalInput").ap()
    tri = nc.dram_tensor("tri", [P, P], BF16, kind="ExternalInput").ap()
    out = nc.dram_tensor("out", [S, D], F32, kind="ExternalOutput").ap()

    with tile.TileContext(nc) as tc:
        with tc.tile_pool(name="const", bufs=1) as cst, \
             tc.tile_pool(name="stage", bufs=3) as stg, \
             tc.tile_pool(name="pt", bufs=5) as ptp, \
             tc.tile_pool(name="small", bufs=2) as sml, \
             tc.tile_pool(name="stgp", bufs=2) as stgp, \
             tc.tile_pool(name="psA", bufs=1, space="PSUM") as psA, \
             tc.tile_pool(name="psC", bufs=1, space="PSUM") as psC:

            mm_ctr = [0]

            def mm_tile(dt=F32):
                i = mm_ctr[0]
                mm_ctr[0] += 1
                return psA.tile([P, 1024], dt, tag=f"s{i % 3}",
                                name=f"mm_s{i % 3}")

            # --- constants (already bf16 from host) ---
            tri_bf = cst.tile([P, P], BF16, tag="tri")
            bq_sb = cst.tile([P, OC], F32, tag="bq")
            bk_sb = cst.tile([P, OC], F32, tag="bk")
            bv_sb = cst.tile([HD, NHC], F32, tag="bv")
            w_sbs = {}
            for name in ("wq", "wk", "wv"):
                w_sbs[name] = cst.tile([P, FC, DC], BF16, tag=name, name=name)
            wo_sb = cst.tile([P, OC, D], BF16, tag="wo")

            def emit_weights():
                nc.gpsimd.dma_start(tri_bf[:], tri[:])
                nc.gpsimd.dma_start(bq_sb[:],
                                    bq.rearrange("(c p) -> p c", p=P))
                nc.gpsimd.dma_start(bk_sb[:],
                                    bk.rearrange("(c p) -> p c", p=P))
                nc.gpsimd.dma_start(bv_sb[:],
                                    bv.rearrange("(h p) -> p h", p=HD))
                for wi, (name, wdram) in enumerate(
                        (("wq", wq), ("wk", wk), ("wv", wv))):
                    w_sb = w_sbs[name]
                    for fc in range(FC):
                        wq_ = nc.sync if (wi + fc) % 2 == 0 else nc.gpsimd
                        wq_.dma_start(w_sb[:, fc, :],
                                      wdram[fc * P:(fc + 1) * P, :])
                for c in range(OC):
                    wq_ = nc.sync if c % 2 == 0 else nc.gpsimd
                    wq_.dma_start(wo_sb[:, c, :], wo[c * P:(c + 1) * P, :])

            # --- per-window tiles ---
            xT_w, qT_w, kT_w, v_w, ctx_w = [], [], [], [], []
            for j in range(NWIN):
                xT_w.append(cst.tile([P, FC, W], BF16, tag=f"xT{j}",
                                     name=f"xT{j}"))
                qT_w.append(cst.tile([P, OC, W], BF16, tag=f"qT{j}",
                                     name=f"qT{j}"))
                kT_w.append(cst.tile([P, OC, W], BF16, tag=f"kT{j}",
                                     name=f"kT{j}"))
                v_w.append(cst.tile([P, WT, NHC, HD + 1], BF16, tag=f"v{j}",
                                    name=f"v{j}"))
                ctx_w.append(cst.tile([P, NPAIR, W], BF16, tag=f"ctx{j}",
                                      name=f"ctx{j}"))
                nc.vector.memset(v_w[j][:, :, :, HD:HD + 1], 1.0)

            eng_ctr = [0]

            def nxt():
                eng_ctr[0] += 1
                return eng_ctr[0]

            def emit_xt(j):
                # --- x^T window j: XBAR DMA transpose straight from DRAM ---
                for t in range(WT):
                    tokc = j * WT + t
                    nc.sync.dma_start_transpose(
                        xT_w[j][:, :, t * P:(t + 1) * P],
                        x[tokc * P:(tokc + 1) * P, :])

            def emit_proj(j, with_xt=True):
                late = j >= 2
                if with_xt:
                    emit_xt(j)

                # --- Q^T / K^T window j (2 outdim chunks per psum slot) ---
                for dst, wname, b_sb in ((qT_w[j], "wq", bq_sb),
                                         (kT_w[j], "wk", bk_sb)):
                    w_sb = w_sbs[wname]
                    for og in range(OC // 2):
                        ps = mm_tile()
                        for half in range(2):
                            oc = og * 2 + half
                            for fc in range(FC):
                                nc.tensor.matmul(
                                    ps[:, half * W:(half + 1) * W],
                                    w_sb[:, fc, oc * P:(oc + 1) * P],
                                    xT_w[j][:, fc, :],
                                    start=(fc == 0), stop=(fc == FC - 1))
                        for half in range(2):
                            oc = og * 2 + half
                            hv = ps[:, half * W:(half + 1) * W]
                            if nxt() % 2 == 0:
                                nc.vector.tensor_scalar(
                                    dst[:, oc, :], hv,
                                    b_sb[:, oc:oc + 1], None, ALU.add)
                            else:
                                nc.scalar.activation(
                                    dst[:, oc, :], hv, AF.Identity,
                                    bias=b_sb[:, oc:oc + 1])

                # --- V window j (2 token chunks per psum slot) ---
                for tg in range(WT // 2):
                    ps = mm_tile()
                    for half in range(2):
                        t = tg * 2 + half
                        for fc in range(FC):
                            nc.tensor.matmul(
                                ps[:, half * W:(half + 1) * W],
                                xT_w[j][:, fc, t * P:(t + 1) * P],
                                w_sbs["wv"][:, fc, :],
                                start=(fc == 0), stop=(fc == FC - 1))
                    dv = ps.rearrange("p (t h n) -> p t h n", t=2, h=NHC)
                    _copy(nc, nxt(), v_w[j][:, tg * 2:tg * 2 + 2, :, 0:HD], dv)

            def emit_attention(j, pairs, stgw=None, rsw=None):
                # --- attention for the given head pairs, window j ---
                skc_hi = WT * (j + 1)
                if stgw is None:
                    stgw = stgp.tile([HD, NHC, W], BF16, tag="stgw",
                                     name="stgw")
                    rsw = sml.tile([NHC, W], F32, tag="rsw", name="rsw")
                for p in pairs:
                    ctx0 = psC.tile([P, W], F32, tag="c0", name="ctx0")
                    ctx1 = psC.tile([P, W], F32, tag="c1", name="ctx1")
                    for skc in range(skc_hi):
                        jk, tk = divmod(skc, WT)
                        rel = skc * P - j * W
                        vs = max(rel, 0)
                        sp = mm_tile()
                        spv = sp.rearrange("p (h n) -> p h n", h=2)
                        nc.tensor.matmul(sp[:, vs:W],
                                         kT_w[jk][0:HD, p, tk * P:(tk + 1) * P],
                                         qT_w[j][0:HD, p, vs:W],
                                         start=True, stop=True)
                        nc.tensor.matmul(sp[:, W + vs:2 * W],
                                         kT_w[jk][HD:P, p, tk * P:(tk + 1) * P],
                                         qT_w[j][HD:P, p, vs:W],
                                         start=True, stop=True)
                        pt = ptp.tile([P, 1024], BF16, tag="pt", name="pt")
                        ptv = pt.rearrange("p (h n) -> p h n", h=2)
                        nc.scalar.activation(ptv[:, :, vs:W], spv[:, :, vs:W],
                                             AF.Exp, scale=SCALE)
                        if rel >= 0:
                            nc.vector.tensor_tensor(
                                ptv[:, :, rel:rel + P], ptv[:, :, rel:rel + P],
                                tri_bf[:, None, :].to_broadcast([P, 2, P]),
                                ALU.mult)
                        st0 = (skc == 0)
                        sp0 = (skc == skc_hi - 1)
                        nc.tensor.matmul(ctx0[0:HD + 1, vs:W],
                                         v_w[jk][:, tk, 2 * p, :],
                                         ptv[:, 0, vs:W], start=st0, stop=sp0)
                        nc.tensor.matmul(ctx1[0:HD + 1, vs:W],
                                         v_w[jk][:, tk, 2 * p + 1, :],
                                         ptv[:, 1, vs:W], start=st0, stop=sp0)

                    # fast drains; rowsums to the window collector
                    for h, ctxp in ((0, ctx0), (1, ctx1)):
                        rw = sml.tile([1, W], F32, tag=f"rw{h}", name="rw")
                        nc.vector.tensor_copy(rw[:], ctxp[HD:HD + 1, :])
                        ri = 2 * (p - pairs[0]) + h
                        rq = nc.sync if (p + h) % 2 == 0 else nc.gpsimd
                        rq.dma_start(rsw[ri:ri + 1, :], rw[:])
                        _copy(nc, nxt(), stgw[:, 2 * p + h, :], ctxp[0:HD, :])

                return stgw, rsw

            def emit_norm(j, stgw, rsw, pairs):
                # batched reciprocal + normalization for the given pairs
                nr = 2 * len(pairs)
                rcf = sml.tile([NHC, W], F32, tag="rcf", name="rcf")
                nc.vector.reciprocal(rcf[0:nr, :], rsw[0:nr, :])
                rcw = sml.tile([NHC, W], BF16, tag="rcw", name="rcw")
                nc.vector.tensor_copy(rcw[0:nr, :], rcf[0:nr, :])
                for p in pairs:
                    for h in range(2):
                        i = 2 * p + h
                        ri = 2 * (p - pairs[0]) + h
                        bc = sml.tile([HD, W], BF16, tag=f"bc{i % 2}",
                                      name="bc")
                        bq_ = nc.sync if i % 2 == 0 else nc.gpsimd
                        bq_.dma_start(
                            bc[:], rcw[ri:ri + 1, None, :].to_broadcast(
                                [1, HD, W]))
                        if h == 0:
                            dst = ctx_w[j][0:HD, p, :]
                            nc.vector.tensor_tensor(dst, stgw[:, i, :], bc[:],
                                                    ALU.mult)
                            if with_bv:
                                nc.vector.tensor_scalar(
                                    dst, dst, bv_sb[:, i:i + 1], None, ALU.add)
                        else:
                            sh = sml.tile([HD, W], BF16, tag="sh", name="sh")
                            nc.vector.tensor_tensor(sh[:], stgw[:, i, :],
                                                    bc[:], ALU.mult)
                            if with_bv:
                                nc.vector.tensor_scalar(
                                    sh[:], sh[:], bv_sb[:, i:i + 1], None,
                                    ALU.add)
                            shq = nc.gpsimd if p % 2 == 0 else nc.sync
                            shq.dma_start(ctx_w[j][HD:P, p, :], sh[:])

            def emit_outproj(j):
                # --- output projection for window j's tokens ---
                for t in range(WT):
                    tokc = j * WT + t
                    ps = mm_tile()
                    for nb in range(2):
                        for pr in range(NPAIR):
                            nc.tensor.matmul(
                                ps[:, nb * W:(nb + 1) * W],
                                ctx_w[j][:, pr, t * P:(t + 1) * P],
                                wo_sb[:, pr, nb * 512:(nb + 1) * 512],
                                start=(pr == 0), stop=(pr == NPAIR - 1))
                    ost = stg.tile([P, D], F32, tag="ostage")
                    _copy(nc, nxt(), ost[:], ps[:])
                    oeng = nc.sync if t % 2 == 0 else nc.gpsimd
                    oeng.dma_start(out[tokc * P:(tokc + 1) * P, :], ost[:])

            emit_xt(0)
            emit_weights()
            emit_proj(0, with_xt=False)
            for j in range(NWIN):
                if j + 1 < NWIN:
                    stgw, rsw = emit_attention(j, list(range(NPAIR)))
                    emit_proj(j + 1)
                    emit_norm(j, stgw, rsw, list(range(NPAIR)))
                else:
                    units = [emit_attention(j, [p]) + ([p],)
                             for p in range(2)]
                    for p in range(2, NPAIR):
                        u = emit_attention(j, [p])
                        emit_norm(j, *units.pop(0))
                        units.append(u + ([p],))
                    for u in units:
                        emit_norm(j, *u)
                emit_outproj(j)

    nc.compile()
    return nc


def make_in_maps(x, Wq, bq, Wk, bk, Wv, bv, Wo):
    BF = ml_dtypes.bfloat16
    # tri[p, f] = 1 where f >= p (keep key p for query f within a diag block)
    tri = np.triu(np.ones((P, P), dtype=np.float32)).astype(BF)
    in_maps = []
    for c in range(8):
        b, g = c // 2, c % 2
        sl = slice(g * DC, (g + 1) * DC)
        in_maps.append({
            "x": np.ascontiguousarray(x[b]).astype(BF),
            "wq": np.ascontiguousarray(Wq[:, sl]).astype(BF),
            "wk": np.ascontiguousarray(Wk[:, sl]).astype(BF),
            "wv": np.ascontiguousarray(Wv[:, sl]).astype(BF),
            "wo": np.ascontiguousarray(Wo[sl, :]).astype(BF),
            "bq": np.ascontiguousarray(bq[sl]),
            "bk": np.ascontiguousarray(bk[sl]),
            "bv": np.ascontiguousarray(bv[sl]),
            "tri": tri,
        })
    return in_maps


_NC_CACHE = {}


def kernel(x, Wq, bq, Wk, bk, Wv, bv, Wo, bo):
    x = np.asarray(x, dtype=np.float32)
    args = [np.asarray(a, dtype=np.float32)
            for a in (Wq, bq, Wk, bk, Wv, bv, Wo, bo)]
    Wq, bq, Wk, bk, Wv, bv, Wo, bo = args
    key = ("nc", x.shape[1], bool(np.any(bv)))
    if key not in _NC_CACHE:
        _NC_CACHE[key] = build_nc(S=x.shape[1], num_devices=8,
                                  with_bv=bool(np.any(bv)))
    nc = _NC_CACHE[key]
    in_maps = make_in_maps(x, Wq, bq, Wk, bk, Wv, bv, Wo)
    res = run_bass_kernel_spmd(nc, in_maps, core_ids=list(range(8)))
    B = x.shape[0]
    out = np.empty_like(x)
    for b in range(B):
        out[b] = res.results[2 * b]["out"] + res.results[2 * b + 1]["out"] + bo
    return out


# revision 36
# speedup vs baseline: 1.1857x; 1.0990x over previous
"""Causal multi-head attention layer (train forward) on 8 Trainium2 NeuronCores.

Sharding: batch (4) x head-group (2 of 8 heads each) -> 8 cores.
Per core (batch b, head group g): project Q^T/K^T [512,S] and V [S,512] from
x_b in bf16 (fp32 PSUM accum), run causal attention head-pair-packed on the PE
array (row tiles at partitions 0/64, one [128,1024] PSUM strip pair per key
chunk), softmax rowsums ride a ones column on V (ctx matmul M=65), the
normalization reciprocal is batched per window on DVE and applied during the
ctx drain, then a partial output projection with this core's Wo row block.
Host pre-casts weights/x to bf16, sums the two partials per batch, adds bo.
"""
import numpy as np
import ml_dtypes

import concourse.bass as bass
import concourse.tile as tile
from concourse import bacc, mybir
from concourse.bass_utils import run_bass_kernel_spmd

F32 = mybir.dt.float32
BF16 = mybir.dt.bfloat16
AF = mybir.ActivationFunctionType
ALU = mybir.AluOpType

P = 128
D = 1024          # model dim
DC = 512          # per-core head dims (8 heads x 64)
HD = 64
NHC = 8           # heads per core
NPAIR = 4         # head pairs per core
FC = D // P       # 8 feature chunks
OC = DC // P      # 4 outdim chunks (= head pairs)
W = 512           # query window (fp32 PSUM bank)
WT = W // P       # token chunks per window
SCALE = 1.0 / 32.0  # 1/sqrt(D)


def _copy(nc, i, out, in_, dve_only=False):
    if dve_only or i % 2 == 0:
        nc.vector.tensor_copy(out, in_)
    else:
        nc.scalar.copy(out, in_)


def build_nc(S=2048, num_devices=8, with_bv=False):
    NWIN = S // W

    nc = bacc.Bacc("TRN2", target_bir_lowering=False, debug=False,
                   num_devices=num_devices)
    x = nc.dram_tensor("x", [S, D], BF16, kind="ExternalInput").ap()
    wq = nc.dram_tensor("wq", [D, DC], BF16, kind="ExternalInput").ap()
    wk = nc.dram_tensor("wk", [D, DC], BF16, kind="ExternalInput").ap()
    wv = nc.dram_tensor("wv", [D, DC], BF16, kind="ExternalInput").ap()
    wo = nc.dram_tensor("wo", [DC, D], BF16, kind="ExternalInput").ap()
    bq = nc.dram_tensor("bq", [DC], F32, kind="ExternalInput").ap()
    bk = nc.dram_tensor("bk", [DC], F32, kind="ExternalInput").ap()
    bv = nc.dram_tensor("bv", [DC], F32, kind="ExternalInput").ap()
    tri = nc.dram_tensor("tri", [P, P], BF16, kind="ExternalInput").ap()
    out = nc.dram_tensor("out", [S, D], F32, kind="ExternalOutput").ap()

    with tile.TileContext(nc) as tc:
        with tc.tile_pool(name="const", bufs=1) as cst, \
             tc.tile_pool(name="stage", bufs=3) as stg, \
             tc.tile_pool(name="pt", bufs=5) as ptp, \
             tc.tile_pool(name="small", bufs=2) as sml, \
             tc.tile_pool(name="stgp", bufs=2) as stgp, \
             tc.tile_pool(name="psA", bufs=1, space="PSUM") as psA, \
             tc.tile_pool(name="psC", bufs=1, space="PSUM") as psC:

            mm_ctr = [0]

            def mm_tile(dt=F32):
                i = mm_ctr[0]
                mm_ctr[0] += 1
                return psA.tile([P, 1024], dt, tag=f"s{i % 3}",
                                name=f"mm_s{i % 3}")

            # --- constants (already bf16 from host) ---
            tri_bf = cst.tile([P, P], BF16, tag="tri")
            bq_sb = cst.tile([P, OC], F32, tag="bq")
            bk_sb = cst.tile([P, OC], F32, tag="bk")
            bv_sb = cst.tile([HD, NHC], F32, tag="bv")
            w_sbs = {}
            for name in ("wq", "wk", "wv"):
                w_sbs[name] = cst.tile([P, FC, DC], BF16, tag=name, name=name)
            wo_sb = cst.tile([P, OC, D], BF16, tag="wo")

            def emit_weights():
                nc.gpsimd.dma_start(tri_bf[:], tri[:])
                nc.gpsimd.dma_start(bq_sb[:],
                                    bq.rearrange("(c p) -> p c", p=P))
                nc.gpsimd.dma_start(bk_sb[:],
                                    bk.rearrange("(c p) -> p c", p=P))
                nc.gpsimd.dma_start(bv_sb[:],
                                    bv.rearrange("(h p) -> p h", p=HD))
                for wi, (name, wdram) in enumerate(
                        (("wq", wq), ("wk", wk), ("wv", wv))):
                    w_sb = w_sbs[name]
                    for fc in range(FC):
                        wq_ = nc.sync if (wi + fc) % 2 == 0 else nc.gpsimd
                        wq_.dma_start(w_sb[:, fc, :],
                                      wdram[fc * P:(fc + 1) * P, :])
                for c in range(OC):
                    wq_ = nc.sync if c % 2 == 0 else nc.gpsimd
                    wq_.dma_start(wo_sb[:, c, :], wo[c * P:(c + 1) * P, :])

            # --- per-window tiles ---
            xT_w, qT_w, kT_w, v_w, ctx_w = [], [], [], [], []
            for j in range(NWIN):
                xT_w.append(cst.tile([P, FC, W], BF16, tag=f"xT{j}",
                                     name=f"xT{j}"))
                qT_w.append(cst.tile([P, OC, W], BF16, tag=f"qT{j}",
                                     name=f"qT{j}"))
                kT_w.append(cst.tile([P, OC, W], BF16, tag=f"kT{j}",
                                     name=f"kT{j}"))
                v_w.append(cst.tile([P, WT, NHC, HD + 1], BF16, tag=f"v{j}",
                                    name=f"v{j}"))
                ctx_w.append(cst.tile([P, NPAIR, W], BF16, tag=f"ctx{j}",
                                      name=f"ctx{j}"))
                nc.vector.memset(v_w[j][:, :, :, HD:HD + 1], 1.0)

            eng_ctr = [0]

            def nxt():
                eng_ctr[0] += 1
                return eng_ctr[0]

            def emit_xt(j):
                # --- x^T window j: XBAR DMA transpose straight from DRAM ---
                for t in range(WT):
                    tokc = j * WT + t
                    nc.sync.dma_start_transpose(
                        xT_w[j][:, :, t * P:(t + 1) * P],
                        x[tokc * P:(tokc + 1) * P, :])

            def emit_proj(j, with_xt=True):
                late = j >= 2
                if with_xt:
                    emit_xt(j)

                # --- Q^T / K^T window j (2 outdim chunks per psum slot) ---
                for dst, wname, b_sb in ((qT_w[j], "wq", bq_sb),
                                         (kT_w[j], "wk", bk_sb)):
                    w_sb = w_sbs[wname]
                    for og in range(OC // 2):
                        ps = mm_tile()
                        for half in range(2):
                            oc = og * 2 + half
                            for fc in range(FC):
                                nc.tensor.matmul(
                                    ps[:, half * W:(half + 1) * W],
                                    w_sb[:, fc, oc * P:(oc + 1) * P],
                                    xT_w[j][:, fc, :],
                                    start=(fc == 0), stop=(fc == FC - 1))
                        for half in range(2):
                            oc = og * 2 + half
                            hv = ps[:, half * W:(half + 1) * W]
                            if nxt() % 2 == 0:
                                nc.vector.tensor_scalar(
                                    dst[:, oc, :], hv,
                                    b_sb[:, oc:oc + 1], None, ALU.add)
                            else:
                                nc.scalar.activation(
                                    dst[:, oc, :], hv, AF.Identity,
                                    bias=b_sb[:, oc:oc + 1])

                # --- V window j (2 token chunks per psum slot) ---
                for tg in range(WT // 2):
                    ps = mm_tile()
                    for half in range(2):
                        t = tg * 2 + half
                        for fc in range(FC):
                            nc.tensor.matmul(
                                ps[:, half * W:(half + 1) * W],
                                xT_w[j][:, fc, t * P:(t + 1) * P],
                                w_sbs["wv"][:, fc, :],
                                start=(fc == 0), stop=(fc == FC - 1))
                    dv = ps.rearrange("p (t h n) -> p t h n", t=2, h=NHC)
                    _copy(nc, nxt(), v_w[j][:, tg * 2:tg * 2 + 2, :, 0:HD], dv)

            def emit_attention(j, pairs, stgw=None, rsw=None):
                # --- attention for the given head pairs, window j ---
                skc_hi = WT * (j + 1)
                if stgw is None:
                    stgw = stgp.tile([HD, NHC, W], BF16, tag="stgw",
                                     name="stgw")
                    rsw = sml.tile([NHC, W], F32, tag="rsw", name="rsw")
                for p in pairs:
                    ctx0 = psC.tile([P, W], F32, tag="c0", name="ctx0")
                    ctx1 = psC.tile([P, W], F32, tag="c1", name="ctx1")
                    for skc in range(skc_hi):
                        jk, tk = divmod(skc, WT)
                        rel = skc * P - j * W
                        vs = max(rel, 0)
                        sp = mm_tile()
                        spv = sp.rearrange("p (h n) -> p h n", h=2)
                        nc.tensor.matmul(sp[:, vs:W],
                                         kT_w[jk][0:HD, p, tk * P:(tk + 1) * P],
                                         qT_w[j][0:HD, p, vs:W],
                                         start=True, stop=True)
                        nc.tensor.matmul(sp[:, W + vs:2 * W],
                                         kT_w[jk][HD:P, p, tk * P:(tk + 1) * P],
                                         qT_w[j][HD:P, p, vs:W],
                                         start=True, stop=True)
                        pt = ptp.tile([P, 1024], BF16, tag="pt", name="pt")
                        ptv = pt.rearrange("p (h n) -> p h n", h=2)
                        nc.scalar.activation(ptv[:, :, vs:W], spv[:, :, vs:W],
                                             AF.Exp, scale=SCALE)
                        if rel >= 0:
                            nc.vector.tensor_tensor(
                                ptv[:, :, rel:rel + P], ptv[:, :, rel:rel + P],
                                tri_bf[:, None, :].to_broadcast([P, 2, P]),
                                ALU.mult)
                        st0 = (skc == 0)
                        sp0 = (skc == skc_hi - 1)
                        nc.tensor.matmul(ctx0[0:HD + 1, vs:W],
                                         v_w[jk][:, tk, 2 * p, :],
                                         ptv[:, 0, vs:W], start=st0, stop=sp0)
                        nc.tensor.matmul(ctx1[0:HD + 1, vs:W],
                                         v_w[jk][:, tk, 2 * p + 1, :],
                                         ptv[:, 1, vs:W], start=st0, stop=sp0)

                    # fast drains; rowsums to the window collector
                    for h, ctxp in ((0, ctx0), (1, ctx1)):
                        rw = sml.tile([1, W], F32, tag=f"rw{h}", name="rw")
                        nc.vector.tensor_copy(rw[:], ctxp[HD:HD + 1, :])
                        ri = 2 * (p - pairs[0]) + h
                        rq = nc.sync if (p + h) % 2 == 0 else nc.gpsimd
                        rq.dma_start(rsw[ri:ri + 1, :], rw[:])
                        _copy(nc, nxt(), stgw[:, 2 * p + h, :], ctxp[0:HD, :])

                return stgw, rsw

            def emit_norm(j, stgw, rsw, pairs):
                # batched reciprocal + normalization for the given pairs
                nr = 2 * len(pairs)
                rcf = sml.tile([NHC, W], F32, tag="rcf", name="rcf")
                nc.vector.reciprocal(rcf[0:nr, :], rsw[0:nr, :])
                rcw = sml.tile([NHC, W], BF16, tag="rcw", name="rcw")
                nc.scalar.copy(rcw[0:nr, :], rcf[0:nr, :])
                for p in pairs:
                    for h in range(2):
                        i = 2 * p + h
                        ri = 2 * (p - pairs[0]) + h
                        bc = sml.tile([HD, W], BF16, tag=f"bc{i % 2}",
                                      name="bc")
                        bq_ = nc.sync if i % 2 == 0 else nc.gpsimd
                        bq_.dma_start(
                            bc[:], rcw[ri:ri + 1, None, :].to_broadcast(
                                [1, HD, W]))
                        if h == 0:
                            dst = ctx_w[j][0:HD, p, :]
                            nc.vector.tensor_tensor(dst, stgw[:, i, :], bc[:],
                                                    ALU.mult)
                            if with_bv:
                                nc.vector.tensor_scalar(
                                    dst, dst, bv_sb[:, i:i + 1], None, ALU.add)
                        else:
                            sh = sml.tile([HD, W], BF16, tag="sh", name="sh")
                            nc.vector.tensor_tensor(sh[:], stgw[:, i, :],
                                                    bc[:], ALU.mult)
                            if with_bv:
                                nc.vector.tensor_scalar(
                                    sh[:], sh[:], bv_sb[:, i:i + 1], None,
                                    ALU.add)
                            shq = nc.gpsimd if p % 2 == 0 else nc.sync
                            shq.dma_start(ctx_w[j][HD:P, p, :], sh[:])

            def emit_outproj(j):
                # --- output projection for window j's tokens ---
                for t in range(WT):
                    tokc = j * WT + t
                    ps = mm_tile()
                    for nb in range(2):
                        for pr in range(NPAIR):
                            nc.tensor.matmul(
                                ps[:, nb * W:(nb + 1) * W],
                                ctx_w[j][:, pr, t * P:(t + 1) * P],
                                wo_sb[:, pr, nb * 512:(nb + 1) * 512],
                                start=(pr == 0), stop=(pr == NPAIR - 1))
                    ost = stg.tile([P, D], F32, tag="ostage")
                    _copy(nc, nxt(), ost[:], ps[:])
                    oeng = nc.sync if t % 2 == 0 else nc.gpsimd
                    oeng.dma_start(out[tokc * P:(tokc + 1) * P, :], ost[:])

            emit_xt(0)
            emit_weights()
            for j in range(1, NWIN):
                emit_xt(j)
            emit_proj(0, with_xt=False)
            for j in range(NWIN):
                if j + 1 < NWIN:
                    stgw, rsw = emit_attention(j, list(range(NPAIR)))
                    emit_proj(j + 1, with_xt=False)
                    emit_norm(j, stgw, rsw, list(range(NPAIR)))
                else:
                    units = [emit_attention(j, [p]) + ([p],)
                             for p in range(2)]
                    for p in range(2, NPAIR):
                        u = emit_attention(j, [p])
                        emit_norm(j, *units.pop(0))
                        units.append(u + ([p],))
                    for u in units:
                        emit_norm(j, *u)
                emit_outproj(j)

    nc.compile()
    return nc


def make_in_maps(x, Wq, bq, Wk, bk, Wv, bv, Wo):
    BF = ml_dtypes.bfloat16
    # tri[p, f] = 1 where f >= p (keep key p for query f within a diag block)
    tri = np.triu(np.ones((P, P), dtype=np.float32)).astype(BF)
    in_maps = []
    for c in range(8):
        b, g = c // 2, c % 2
        sl = slice(g * DC, (g + 1) * DC)
        in_maps.append({
            "x": np.ascontiguousarray(x[b]).astype(BF),
            "wq": np.ascontiguousarray(Wq[:, sl]).astype(BF),
            "wk": np.ascontiguousarray(Wk[:, sl]).astype(BF),
            "wv": np.ascontiguousarray(Wv[:, sl]).astype(BF),
            "wo": np.ascontiguousarray(Wo[sl, :]).astype(BF),
            "bq": np.ascontiguousarray(bq[sl]),
            "bk": np.ascontiguousarray(bk[sl]),
            "bv": np.ascontiguousarray(bv[sl]),
            "tri": tri,
        })
    return in_maps


_NC_CACHE = {}


def kernel(x, Wq, bq, Wk, bk, Wv, bv, Wo, bo):
    x = np.asarray(x, dtype=np.float32)
    args = [np.asarray(a, dtype=np.float32)
            for a in (Wq, bq, Wk, bk, Wv, bv, Wo, bo)]
    Wq, bq, Wk, bk, Wv, bv, Wo, bo = args
    key = ("nc", x.shape[1], bool(np.any(bv)))
    if key not in _NC_CACHE:
        _NC_CACHE[key] = build_nc(S=x.shape[1], num_devices=8,
                                  with_bv=bool(np.any(bv)))
    nc = _NC_CACHE[key]
    in_maps = make_in_maps(x, Wq, bq, Wk, bk, Wv, bv, Wo)
    res = run_bass_kernel_spmd(nc, in_maps, core_ids=list(range(8)))
    B = x.shape[0]
    out = np.empty_like(x)
    for b in range(B):
        out[b] = res.results[2 * b]["out"] + res.results[2 * b + 1]["out"] + bo
    return out


# revision 37
# speedup vs baseline: 1.2416x; 1.0471x over previous
"""Causal multi-head attention layer (train forward) on 8 Trainium2 NeuronCores.

Sharding: batch (4) x head-group (2 of 8 heads each) -> 8 cores.
Per core (batch b, head group g): project Q^T/K^T [512,S] and V [S,512] from
x_b in bf16 (fp32 PSUM accum), run causal attention head-pair-packed on the PE
array (row tiles at partitions 0/64, one [128,1024] PSUM strip pair per key
chunk), softmax rowsums ride a ones column on V (ctx matmul M=65), the
normalization reciprocal is batched per window on DVE and applied during the
ctx drain, then a partial output projection with this core's Wo row block.
Host pre-casts weights/x to bf16, sums the two partials per batch, adds bo.
"""
import numpy as np
import ml_dtypes

import concourse.bass as bass
import concourse.tile as tile
from concourse import bacc, mybir
from concourse.bass_utils import run_bass_kernel_spmd

F32 = mybir.dt.float32
BF16 = mybir.dt.bfloat16
AF = mybir.ActivationFunctionType
ALU = mybir.AluOpType

P = 128
D = 1024          # model dim
DC = 512          # per-core head dims (8 heads x 64)
HD = 64
NHC = 8           # heads per core
NPAIR = 4         # head pairs per core
FC = D // P       # 8 feature chunks
OC = DC // P      # 4 outdim chunks (= head pairs)
W = 512           # query window (fp32 PSUM bank)
WT = W // P       # token chunks per window
SCALE = 1.0 / 32.0  # 1/sqrt(D)


def _copy(nc, i, out, in_, dve_only=False):
    if dve_only or i % 2 == 0:
        nc.vector.tensor_copy(out, in_)
    else:
        nc.scalar.copy(out, in_)


def build_nc(S=2048, num_devices=8, with_bv=False):
    NWIN = S // W

    nc = bacc.Bacc("TRN2", target_bir_lowering=False, debug=False,
                   num_devices=num_devices)
    x = nc.dram_tensor("x", [S, D], BF16, kind="ExternalInput").ap()
    wq = nc.dram_tensor("wq", [D, DC], BF16, kind="ExternalInput").ap()
    wk = nc.dram_tensor("wk", [D, DC], BF16, kind="ExternalInput").ap()
    wv = nc.dram_tensor("wv", [D, DC], BF16, kind="ExternalInput").ap()
    wo = nc.dram_tensor("wo", [DC, D], BF16, kind="ExternalInput").ap()
    bq = nc.dram_tensor("bq", [DC], F32, kind="ExternalInput").ap()
    bk = nc.dram_tensor("bk", [DC], F32, kind="ExternalInput").ap()
    bv = nc.dram_tensor("bv", [DC], F32, kind="ExternalInput").ap()
    tri = nc.dram_tensor("tri", [P, P], BF16, kind="ExternalInput").ap()
    out = nc.dram_tensor("out", [S, D], F32, kind="ExternalOutput").ap()

    with tile.TileContext(nc) as tc:
        with tc.tile_pool(name="const", bufs=1) as cst, \
             tc.tile_pool(name="stage", bufs=3) as stg, \
             tc.tile_pool(name="pt", bufs=5) as ptp, \
             tc.tile_pool(name="small", bufs=2) as sml, \
             tc.tile_pool(name="stgp", bufs=2) as stgp, \
             tc.tile_pool(name="psA", bufs=1, space="PSUM") as psA, \
             tc.tile_pool(name="psC", bufs=1, space="PSUM") as psC:

            mm_ctr = [0]

            def mm_tile(dt=F32):
                i = mm_ctr[0]
                mm_ctr[0] += 1
                return psA.tile([P, 1024], dt, tag=f"s{i % 3}",
                                name=f"mm_s{i % 3}")

            # --- constants (already bf16 from host) ---
            tri_bf = cst.tile([P, P], BF16, tag="tri")
            bq_sb = cst.tile([P, OC], F32, tag="bq")
            bk_sb = cst.tile([P, OC], F32, tag="bk")
            bv_sb = cst.tile([HD, NHC], F32, tag="bv")
            w_sbs = {}
            for name in ("wq", "wk", "wv"):
                w_sbs[name] = cst.tile([P, FC, DC], BF16, tag=name, name=name)
            wo_sb = cst.tile([P, OC, D], BF16, tag="wo")

            def emit_weights():
                nc.gpsimd.dma_start(tri_bf[:], tri[:])
                nc.gpsimd.dma_start(bq_sb[:],
                                    bq.rearrange("(c p) -> p c", p=P))
                nc.gpsimd.dma_start(bk_sb[:],
                                    bk.rearrange("(c p) -> p c", p=P))
                nc.gpsimd.dma_start(bv_sb[:],
                                    bv.rearrange("(h p) -> p h", p=HD))
                for wi, (name, wdram) in enumerate(
                        (("wq", wq), ("wk", wk), ("wv", wv))):
                    w_sb = w_sbs[name]
                    for fc in range(FC):
                        wq_ = nc.sync if (wi + fc) % 2 == 0 else nc.gpsimd
                        wq_.dma_start(w_sb[:, fc, :],
                                      wdram[fc * P:(fc + 1) * P, :])
                for c in range(OC):
                    wq_ = nc.sync if c % 2 == 0 else nc.gpsimd
                    wq_.dma_start(wo_sb[:, c, :], wo[c * P:(c + 1) * P, :])

            # --- per-window tiles ---
            xT_w, qT_w, kT_w, v_w, ctx_w = [], [], [], [], []
            for j in range(NWIN):
                xT_w.append(cst.tile([P, FC, W], BF16, tag=f"xT{j}",
                                     name=f"xT{j}"))
                qT_w.append(cst.tile([P, OC, W], BF16, tag=f"qT{j}",
                                     name=f"qT{j}"))
                kT_w.append(cst.tile([P, OC, W], BF16, tag=f"kT{j}",
                                     name=f"kT{j}"))
                v_w.append(cst.tile([P, WT, NHC, HD + 1], BF16, tag=f"v{j}",
                                    name=f"v{j}"))
                ctx_w.append(cst.tile([P, NPAIR, W], BF16, tag=f"ctx{j}",
                                      name=f"ctx{j}"))
                nc.vector.memset(v_w[j][:, :, :, HD:HD + 1], 1.0)

            eng_ctr = [0]

            def nxt():
                eng_ctr[0] += 1
                return eng_ctr[0]

            def emit_xt(j):
                # --- x^T window j: XBAR DMA transpose straight from DRAM ---
                for t in range(WT):
                    tokc = j * WT + t
                    nc.sync.dma_start_transpose(
                        xT_w[j][:, :, t * P:(t + 1) * P],
                        x[tokc * P:(tokc + 1) * P, :])

            def emit_proj(j, with_xt=True):
                late = j >= 2
                if with_xt:
                    emit_xt(j)

                # --- Q^T / K^T window j (2 outdim chunks per psum slot) ---
                for dst, wname, b_sb in ((qT_w[j], "wq", bq_sb),
                                         (kT_w[j], "wk", bk_sb)):
                    w_sb = w_sbs[wname]
                    for og in range(OC // 2):
                        ps = mm_tile()
                        for half in range(2):
                            oc = og * 2 + half
                            for fc in range(FC):
                                nc.tensor.matmul(
                                    ps[:, half * W:(half + 1) * W],
                                    w_sb[:, fc, oc * P:(oc + 1) * P],
                                    xT_w[j][:, fc, :],
                                    start=(fc == 0), stop=(fc == FC - 1))
                        for half in range(2):
                            oc = og * 2 + half
                            hv = ps[:, half * W:(half + 1) * W]
                            if nxt() % 2 == 0:
                                nc.vector.tensor_scalar(
                                    dst[:, oc, :], hv,
                                    b_sb[:, oc:oc + 1], None, ALU.add)
                            else:
                                nc.scalar.activation(
                                    dst[:, oc, :], hv, AF.Identity,
                                    bias=b_sb[:, oc:oc + 1])

                # --- V window j (2 token chunks per psum slot) ---
                for tg in range(WT // 2):
                    ps = mm_tile()
                    for half in range(2):
                        t = tg * 2 + half
                        for fc in range(FC):
                            nc.tensor.matmul(
                                ps[:, half * W:(half + 1) * W],
                                xT_w[j][:, fc, t * P:(t + 1) * P],
                                w_sbs["wv"][:, fc, :],
                                start=(fc == 0), stop=(fc == FC - 1))
                    dv = ps.rearrange("p (t h n) -> p t h n", t=2, h=NHC)
                    _copy(nc, nxt(), v_w[j][:, tg * 2:tg * 2 + 2, :, 0:HD], dv)

            def emit_attention(j, pairs, stgw=None, rsw=None):
                # --- attention for the given head pairs, window j ---
                skc_hi = WT * (j + 1)
                if stgw is None:
                    stgw = stgp.tile([HD, NHC, W], BF16, tag="stgw",
                                     name="stgw")
                    rsw = sml.tile([NHC, W], F32, tag="rsw", name="rsw")
                for p in pairs:
                    ctx0 = psC.tile([P, W], F32, tag="c0", name="ctx0")
                    ctx1 = psC.tile([P, W], F32, tag="c1", name="ctx1")
                    for skc in range(skc_hi):
                        jk, tk = divmod(skc, WT)
                        rel = skc * P - j * W
                        vs = max(rel, 0)
                        sp = mm_tile()
                        spv = sp.rearrange("p (h n) -> p h n", h=2)
                        nc.tensor.matmul(sp[:, vs:W],
                                         kT_w[jk][0:HD, p, tk * P:(tk + 1) * P],
                                         qT_w[j][0:HD, p, vs:W],
                                         start=True, stop=True)
                        nc.tensor.matmul(sp[:, W + vs:2 * W],
                                         kT_w[jk][HD:P, p, tk * P:(tk + 1) * P],
                                         qT_w[j][HD:P, p, vs:W],
                                         start=True, stop=True)
                        pt = ptp.tile([P, 1024], BF16, tag="pt", name="pt")
                        ptv = pt.rearrange("p (h n) -> p h n", h=2)
                        nc.scalar.activation(ptv[:, :, vs:W], spv[:, :, vs:W],
                                             AF.Exp, scale=SCALE)
                        if rel >= 0:
                            nc.vector.tensor_tensor(
                                ptv[:, :, rel:rel + P], ptv[:, :, rel:rel + P],
                                tri_bf[:, None, :].to_broadcast([P, 2, P]),
                                ALU.mult)
                        st0 = (skc == 0)
                        sp0 = (skc == skc_hi - 1)
                        nc.tensor.matmul(ctx0[0:HD + 1, vs:W],
                                         v_w[jk][:, tk, 2 * p, :],
                                         ptv[:, 0, vs:W], start=st0, stop=sp0)
                        nc.tensor.matmul(ctx1[0:HD + 1, vs:W],
                                         v_w[jk][:, tk, 2 * p + 1, :],
                                         ptv[:, 1, vs:W], start=st0, stop=sp0)

                    # fast drains; rowsums to the window collector
                    for h, ctxp in ((0, ctx0), (1, ctx1)):
                        rw = sml.tile([1, W], F32, tag=f"rw{h}", name="rw")
                        nc.vector.tensor_copy(rw[:], ctxp[HD:HD + 1, :])
                        ri = 2 * (p - pairs[0]) + h
                        rq = nc.sync if (p + h) % 2 == 0 else nc.gpsimd
                        rq.dma_start(rsw[ri:ri + 1, :], rw[:])
                        _copy(nc, nxt(), stgw[:, 2 * p + h, :], ctxp[0:HD, :])

                return stgw, rsw

            def emit_norm(j, stgw, rsw, pairs):
                # batched reciprocal + normalization for the given pairs
                nr = 2 * len(pairs)
                rcf = sml.tile([NHC, W], F32, tag="rcf", name="rcf")
                nc.vector.reciprocal(rcf[0:nr, :], rsw[0:nr, :])
                rcw = sml.tile([NHC, W], BF16, tag="rcw", name="rcw")
                nc.vector.tensor_copy(rcw[0:nr, :], rcf[0:nr, :])
                for p in pairs:
                    for h in range(2):
                        i = 2 * p + h
                        ri = 2 * (p - pairs[0]) + h
                        bc = sml.tile([HD, W], BF16, tag=f"bc{i % 2}",
                                      name="bc")
                        bq_ = nc.sync if i % 2 == 0 else nc.gpsimd
                        bq_.dma_start(
                            bc[:], rcw[ri:ri + 1, None, :].to_broadcast(
                                [1, HD, W]))
                        if h == 0:
                            dst = ctx_w[j][0:HD, p, :]
                            nc.vector.tensor_tensor(dst, stgw[:, i, :], bc[:],
                                                    ALU.mult)
                            if with_bv:
                                nc.vector.tensor_scalar(
                                    dst, dst, bv_sb[:, i:i + 1], None, ALU.add)
                        else:
                            sh = sml.tile([HD, W], BF16, tag="sh", name="sh")
                            nc.vector.tensor_tensor(sh[:], stgw[:, i, :],
                                                    bc[:], ALU.mult)
                            if with_bv:
                                nc.vector.tensor_scalar(
                                    sh[:], sh[:], bv_sb[:, i:i + 1], None,
                                    ALU.add)
                            shq = nc.gpsimd if p % 2 == 0 else nc.sync
                            shq.dma_start(ctx_w[j][HD:P, p, :], sh[:])

            def emit_outproj(j):
                # --- output projection for window j's tokens ---
                for t in range(WT):
                    tokc = j * WT + t
                    ps = mm_tile()
                    for nb in range(2):
                        for pr in range(NPAIR):
                            nc.tensor.matmul(
                                ps[:, nb * W:(nb + 1) * W],
                                ctx_w[j][:, pr, t * P:(t + 1) * P],
                                wo_sb[:, pr, nb * 512:(nb + 1) * 512],
                                start=(pr == 0), stop=(pr == NPAIR - 1))
                    ost = stg.tile([P, D], F32, tag="ostage")
                    _copy(nc, nxt(), ost[:], ps[:])
                    oeng = nc.sync if t % 2 == 0 else nc.gpsimd
                    oeng.dma_start(out[tokc * P:(tokc + 1) * P, :], ost[:])

            emit_xt(0)
            emit_weights()
            emit_proj(0, with_xt=False)
            for j in range(NWIN):
                if j + 1 < NWIN:
                    stgw, rsw = emit_attention(j, list(range(NPAIR)))
                    emit_proj(j + 1)
                    emit_norm(j, stgw, rsw, list(range(NPAIR)))
                else:
                    units = [emit_attention(j, [p]) + ([p],)
                             for p in range(2)]
                    for p in range(2, NPAIR):
                        u = emit_attention(j, [p])
                        emit_norm(j, *units.pop(0))
                        units.append(u + ([p],))
                    for u in units:
                        emit_norm(j, *u)
                emit_outproj(j)

    nc.compile()
    return nc


def make_in_maps(x, Wq, bq, Wk, bk, Wv, bv, Wo):
    BF = ml_dtypes.bfloat16
    # tri[p, f] = 1 where f >= p (keep key p for query f within a diag block)
    tri = np.triu(np.ones((P, P), dtype=np.float32)).astype(BF)
    in_maps = []
    for c in range(8):
        b, g = c // 2, c % 2
        sl = slice(g * DC, (g + 1) * DC)
        in_maps.append({
            "x": np.ascontiguousarray(x[b]).astype(BF),
            "wq": np.ascontiguousarray(Wq[:, sl]).astype(BF),
            "wk": np.ascontiguousarray(Wk[:, sl]).astype(BF),
            "wv": np.ascontiguousarray(Wv[:, sl]).astype(BF),
            "wo": np.ascontiguousarray(Wo[sl, :]).astype(BF),
            "bq": np.ascontiguousarray(bq[sl]),
            "bk": np.ascontiguousarray(bk[sl]),
            "bv": np.ascontiguousarray(bv[sl]),
            "tri": tri,
        })
    return in_maps


_NC_CACHE = {}


def kernel(x, Wq, bq, Wk, bk, Wv, bv, Wo, bo):
    x = np.asarray(x, dtype=np.float32)
    args = [np.asarray(a, dtype=np.float32)
            for a in (Wq, bq, Wk, bk, Wv, bv, Wo, bo)]
    Wq, bq, Wk, bk, Wv, bv, Wo, bo = args
    key = ("nc", x.shape[1], bool(np.any(bv)))
    if key not in _NC_CACHE:
        _NC_CACHE[key] = build_nc(S=x.shape[1], num_devices=8,
                                  with_bv=bool(np.any(bv)))
    nc = _NC_CACHE[key]
    in_maps = make_in_maps(x, Wq, bq, Wk, bk, Wv, bv, Wo)
    res = run_bass_kernel_spmd(nc, in_maps, core_ids=list(range(8)))
    B = x.shape[0]
    out = np.empty_like(x)
    for b in range(B):
        out[b] = res.results[2 * b]["out"] + res.results[2 * b + 1]["out"] + bo
    return out
